# revision 9
# baseline (speedup 1.0000x reference)
"""Distributed Trainium2 kernel for LayerNorm -> biased multi-head attention -> out-proj.

Problem shapes (hardcoded):
  x        [4, 2048, 1024] f32
  attn_bias[16, 2048, 2048] f32
  ln_g/ln_b[1024] f32
  Wq       [1024, 1024] f32
  Wkv      [1024, 2048] f32
  Wout     [1024, 1024] f32
  out      [4, 2048, 1024] f32

Sharding: sequence-sharded over 8 cores. Core r owns query rows n in
[r*256, (r+1)*256) for every batch. Each core computes LN + q/k/v for its
rows, AllGathers k^T (inner-major) and v (token-major, with embedded
ones-columns for the softmax denominator) in bf16, then runs attention for
all 16 heads over its local queries and writes its slice of the output.
Softmax uses exp(sim)*exp(bias) factorization (no max subtraction -- values
are small); the denominator comes from the ones-column of V in the attn@v
matmul. PSUM accumulator banks are zero-initialized with a single
start=True zero-weight matmul so the per-(parity,b) accumulation groups
sharing a bank never clear each other's has_written bits.
"""

import numpy as np

CORES = 8
B = 4
N = 2048
NLOC = N // CORES          # 256
ROWS = B * NLOC            # 1024 local query rows (row = b*NLOC + q)
DIM = 1024
H = 16
D = 64
PAIRS = H // 2             # head pairs
KC = N // 128              # 16 kv chunks of 128 tokens per batch
VW = PAIRS * 130           # v row width: per pair [hA(64) 1 | hB(64) 1]
KELEMS = ROWS * DIM        # k^T block elements
VELEMS = ROWS * VW         # v block elements
KVELEMS = KELEMS + VELEMS  # per-rank AllGather payload (elements, bf16)
SCALE = D ** -0.5
EPS = 1e-5

_CACHE = {}


def _build_nc():
    import contextlib
    import concourse.bass as bass
    import concourse.bacc as bacc
    import concourse.tile as tile
    import concourse.mybir as mybir
    from concourse import masks

    f32 = mybir.dt.float32
    bf16 = mybir.dt.bfloat16
    AF = mybir.ActivationFunctionType
    ALU = mybir.AluOpType

    nc = bacc.Bacc("TRN2", target_bir_lowering=False, debug=False,
                   num_devices=CORES)

    x_in = nc.dram_tensor("x", [ROWS, DIM], f32, kind="ExternalInput")
    bias_in = nc.dram_tensor("attn_bias", [H, NLOC, N], f32, kind="ExternalInput")
    ln_g = nc.dram_tensor("ln_g", [DIM], f32, kind="ExternalInput")
    ln_b = nc.dram_tensor("ln_b", [DIM], f32, kind="ExternalInput")
    wq_in = nc.dram_tensor("Wq", [DIM, DIM], f32, kind="ExternalInput")
    wkv_in = nc.dram_tensor("Wkv", [DIM, 2 * DIM], f32, kind="ExternalInput")
    wout_in = nc.dram_tensor("Wout", [DIM, DIM], f32, kind="ExternalInput")
    out_ext = nc.dram_tensor("out", [ROWS, DIM], f32, kind="ExternalOutput")

    with tile.TileContext(nc) as tc, contextlib.ExitStack() as top:
        # ------------------------------------------------------------------
        # DRAM scratch
        dram = top.enter_context(tc.tile_pool(name="dram", bufs=1, space="DRAM"))
        kv_local = dram.tile([KVELEMS], bf16, name="kv_local")
        kv_full = dram.tile([CORES * KVELEMS], bf16, name="kv_full",
                            addr_space="Shared")
        eb_dram = dram.tile([H, NLOC, N], bf16, name="eb_dram")
        biasT_dram = dram.tile([KC, 128, H * NLOC], bf16, name="biasT_dram")

        # ------------------------------------------------------------------
        # Constants
        cpool = top.enter_context(tc.tile_pool(name="consts", bufs=1))
        identity = cpool.tile([128, 128], f32, name="identity")
        masks.make_identity(nc, identity[:])
        eps_t = cpool.tile([128, 1], f32, name="eps_t")
        nc.vector.memset(eps_t[:], EPS)
        g_t = cpool.tile([128, DIM], f32, name="g_t")
        b_t = cpool.tile([128, DIM], f32, name="b_t")
        nc.gpsimd.dma_start(
            out=g_t[:],
            in_=bass.AP(tensor=ln_g.ap().tensor, offset=0, ap=[[0, 128], [1, DIM]]))
        nc.gpsimd.dma_start(
            out=b_t[:],
            in_=bass.AP(tensor=ln_b.ap().tensor, offset=0, ap=[[0, 128], [1, DIM]]))
        # selector for denominator broadcast: sel[h, j] = 1 iff j//64 == h
        sel = cpool.tile([H, H * D], f32, name="sel")
        ones64 = cpool.tile([32, D], f32, name="ones64")
        nc.gpsimd.memset(sel[:], 0.0)
        nc.gpsimd.memset(ones64[:], 1.0)
        for h in range(H):
            nc.gpsimd.dma_start(sel[h:h + 1, h * D:(h + 1) * D], ones64[0:1, :])
        zeros65 = cpool.tile([128, 65], bf16, name="zeros65")
        nc.vector.memset(zeros65[:], 0.0)
        denom_asm = cpool.tile([H, ROWS], f32, name="denom_asm")

        # ------------------------------------------------------------------
        # Persistent pools (live until the end; LIFO with the top stack)
        qt_pool = top.enter_context(tc.tile_pool(name="qT", bufs=1))
        asm_pool = top.enter_context(tc.tile_pool(name="asm", bufs=1))
        asm = [asm_pool.tile([128, ROWS], bf16, name=f"asm{i}") for i in range(8)]

        # ------------------------------------------------------------------
        # Phase 1: LayerNorm (rows on partitions) -> xn f32 in place
        ln_pool = contextlib.ExitStack()
        xpool = ln_pool.enter_context(tc.tile_pool(name="x", bufs=1))
        spool = ln_pool.enter_context(tc.tile_pool(name="stats", bufs=1))
        x_t = []
        for s in range(8):
            xt = xpool.tile([128, DIM], f32, name=f"x{s}")
            nc.sync.dma_start(xt[:], x_in[s * 128:(s + 1) * 128, :])
            stats = spool.tile([128, 2, 6], f32, name=f"st{s}")
            mv = spool.tile([128, 2], f32, name=f"mv{s}")
            for g in range(2):
                nc.vector.bn_stats(stats[:, g], xt[:, g * 512:(g + 1) * 512])
            nc.vector.bn_aggr(mv[:], stats[:])
            # rstd = 1/sqrt(var + eps)
            nc.scalar.activation(mv[:, 1:2], mv[:, 1:2], AF.Sqrt,
                                 bias=eps_t[:, 0:1], scale=1.0)
            nc.vector.reciprocal(mv[:, 1:2], mv[:, 1:2])
            nc.vector.tensor_scalar(out=xt[:], in0=xt[:],
                                    scalar1=mv[:, 0:1], scalar2=mv[:, 1:2],
                                    op0=ALU.subtract, op1=ALU.mult)
            nc.vector.tensor_mul(xt[:], xt[:], g_t[:])
            nc.vector.tensor_add(xt[:], xt[:], b_t[:])
            x_t.append(xt)

        # ------------------------------------------------------------------
        # Phase 2: transpose xn -> xnT bf16 [dim-part, row-free]
        xnt_stack = contextlib.ExitStack()
        xnt_pool = xnt_stack.enter_context(tc.tile_pool(name="xnT", bufs=1))
        tr_stack = contextlib.ExitStack()
        tr_pool = tr_stack.enter_context(
            tc.tile_pool(name="trps", bufs=2, space="PSUM"))
        xnT = []
        for t in range(8):
            ps = tr_pool.tile([128, ROWS], f32, name="trp")
            for s in range(8):
                nc.tensor.transpose(ps[:, s * 128:(s + 1) * 128],
                                    x_t[s][:, t * 128:(t + 1) * 128],
                                    identity[:])
            xt_b = xnt_pool.tile([128, ROWS], bf16, name=f"xnT{t}")
            nc.vector.tensor_copy(xt_b[:], ps[:])
            xnT.append(xt_b)
        tr_stack.close()

        # ------------------------------------------------------------------
        # Phase 3: exp(bias) -> eb_dram -> big xbar transposes -> biasT_dram
        bias_stack = contextlib.ExitStack()
        bpool = bias_stack.enter_context(tc.tile_pool(name="biasin", bufs=2))
        epool = bias_stack.enter_context(tc.tile_pool(name="ebias", bufs=2))
        for h in range(H):
            for qh in range(2):
                bt = bpool.tile([128, N], f32, name="bi")
                nc.scalar.dma_start(bt[:], bias_in[h, qh * 128:(qh + 1) * 128, :])
                et = epool.tile([128, N], bf16, name="eb")
                nc.scalar.activation(et[:], bt[:], AF.Exp)
                nc.scalar.dma_start(eb_dram[h, qh * 128:(qh + 1) * 128, :], et[:])
        # transpose eb_dram viewed as [H*NLOC, N] column-chunk by column-chunk
        ebd_flat = eb_dram[:]
        for c in range(KC):
            st = epool.tile([128, H * NLOC], bf16, name="ebtstage")
            nc.sync.dma_start(
                out=st[:],
                in_=bass.AP(tensor=ebd_flat.tensor,
                            offset=ebd_flat.offset + c * 128,
                            ap=[[N, H * NLOC], [1, 128]]),
                transpose=True)
            nc.sync.dma_start(biasT_dram[c], st[:])
        bias_stack.close()

        # ------------------------------------------------------------------
        # Phase 4: QKV projections (bf16) + kv bounce + AllGather
        w_stack = contextlib.ExitStack()
        wq_pool = w_stack.enter_context(tc.tile_pool(name="wq", bufs=1))
        wkv_pool = w_stack.enter_context(tc.tile_pool(name="wkv", bufs=1))
        wq_bf, wkv_bf = [], []
        for t in range(8):
            wqt = wq_pool.tile([128, DIM], bf16, name=f"wq{t}")
            nc.gpsimd.dma_start(wqt[:], wq_in[t * 128:(t + 1) * 128, :])
            wq_bf.append(wqt)
            wkt = wkv_pool.tile([128, 2 * DIM], bf16, name=f"wkv{t}")
            nc.gpsimd.dma_start(wkt[:], wkv_in[t * 128:(t + 1) * 128, :])
            wkv_bf.append(wkt)

        qkv_psum_stack = contextlib.ExitStack()
        qkv_psum = qkv_psum_stack.enter_context(
            tc.tile_pool(name="qkvp", bufs=2, space="PSUM"))
        stage_stack = contextlib.ExitStack()
        stage_pool = stage_stack.enter_context(tc.tile_pool(name="kvstage", bufs=4))
        vstage_pool = stage_stack.enter_context(tc.tile_pool(name="vstage", bufs=4))

        kvl = kv_local[:]
        KVL_T = kvl.tensor

        qT = []
        for mi in range(8):
            ps = qkv_psum.tile([128, ROWS], f32, name="qkvps")
            for ki in range(8):
                for nh in range(2):
                    nc.tensor.matmul(ps[:, nh * 512:(nh + 1) * 512],
                                     wq_bf[ki][:, mi * 128:(mi + 1) * 128],
                                     xnT[ki][:, nh * 512:(nh + 1) * 512],
                                     start=(ki == 0), stop=(ki == 7))
            qtile = qt_pool.tile([128, ROWS], bf16, name=f"qT{mi}")
            nc.vector.tensor_scalar_mul(qtile[:], ps[:], SCALE)
            qT.append(qtile)

        for mi in range(8):
            ps = qkv_psum.tile([128, ROWS], f32, name="qkvps")
            for ki in range(8):
                for nh in range(2):
                    nc.tensor.matmul(ps[:, nh * 512:(nh + 1) * 512],
                                     wkv_bf[ki][:, mi * 128:(mi + 1) * 128],
                                     xnT[ki][:, nh * 512:(nh + 1) * 512],
                                     start=(ki == 0), stop=(ki == 7))
            kst = stage_pool.tile([128, ROWS], bf16, name="kvst")
            nc.vector.tensor_copy(kst[:], ps[:])
            nc.sync.dma_start(
                out=bass.AP(tensor=KVL_T, offset=kvl.offset + mi * 128 * DIM,
                            ap=[[DIM, 128], [1, DIM]]),
                in_=kst[:])

        for s in range(8):
            ps = qkv_psum.tile([128, DIM], f32, name="qkvps")
            for ki in range(8):
                for nh in range(2):
                    nc.tensor.matmul(ps[:, nh * 512:(nh + 1) * 512],
                                     xnT[ki][:, s * 128:(s + 1) * 128],
                                     wkv_bf[ki][:, DIM + nh * 512:DIM + (nh + 1) * 512],
                                     start=(ki == 0), stop=(ki == 7))
            vst = vstage_pool.tile([128, VW], bf16, name="vst")
            nc.vector.memset(vst[:], 1.0)
            vsb = vst[:]
            for parity in range(2):
                nc.vector.tensor_copy(
                    bass.AP(tensor=vsb.tensor,
                            offset=vsb.offset + parity * 65,
                            ap=[vsb.ap[0], [130, PAIRS], [1, D]]),
                    ps[:].rearrange("p (i2 x) -> p i2 x", i2=8)[
                        :, :, parity * 64:parity * 64 + 64])
            nc.sync.dma_start(
                out=bass.AP(tensor=KVL_T,
                            offset=kvl.offset + KELEMS + s * 128 * VW,
                            ap=[[VW, 128], [1, VW]]),
                in_=vst[:])

        stage_stack.close()
        qkv_psum_stack.close()
        w_stack.close()
        xnt_stack.close()
        ln_pool.close()

        nc.gpsimd.collective_compute(
            "AllGather",
            mybir.AluOpType.bypass,
            replica_groups=[list(range(CORES))],
            ins=[kv_local[:].opt()],
            outs=[kv_full[:].opt()],
        )

        # ------------------------------------------------------------------
        # Phase 5: attention over head pairs (software-pipelined over chunks)
        kvf = kv_full[:]
        KVF_T = kvf.tensor
        bTd = biasT_dram[:]
        BTD_T = bTd.tensor

        attn_stack = contextlib.ExitStack()
        kt_pool = attn_stack.enter_context(tc.tile_pool(name="kT", bufs=2))
        vt_pool = attn_stack.enter_context(tc.tile_pool(name="vt", bufs=2))
        ebt_pool = attn_stack.enter_context(tc.tile_pool(name="ebT", bufs=4))
        ae_pool = attn_stack.enter_context(tc.tile_pool(name="ae", bufs=6))
        den_pool = attn_stack.enter_context(tc.tile_pool(name="den", bufs=2))
        stag_pool = attn_stack.enter_context(tc.tile_pool(name="ostag", bufs=2))
        sim_psum = attn_stack.enter_context(
            tc.tile_pool(name="simp", bufs=2, space="PSUM"))
        out_psum = attn_stack.enter_context(
            tc.tile_pool(name="outp", bufs=4, space="PSUM"))

        for i in range(PAIRS):
            # k^T for the head pair: [128 (2 heads x 64 d), B*N] bf16
            kt = kt_pool.tile([128, B * N], bf16, name="kt")
            ktd = kt[:]
            for b in range(B):
                nc.sync.dma_start(
                    out=bass.AP(tensor=ktd.tensor, offset=ktd.offset + b * N,
                                ap=[ktd.ap[0], [NLOC, CORES], [1, NLOC]]),
                    in_=bass.AP(tensor=KVF_T,
                                offset=kvf.offset + i * 128 * DIM + b * NLOC,
                                ap=[[DIM, 128], [KVELEMS, CORES], [1, NLOC]]))
            # v (with embedded ones cols) for both heads of the pair
            vt = vt_pool.tile([128, B * KC * 130], bf16, name="vt")
            vb = vt[:]
            for b in range(B):
                for c2 in range(2):
                    nc.sync.dma_start(
                        out=bass.AP(tensor=vb.tensor,
                                    offset=(vb.offset + b * KC * 130 + c2 * 130),
                                    ap=[vb.ap[0], [260, CORES], [1, 130]]),
                        in_=bass.AP(tensor=KVF_T,
                                    offset=(kvf.offset + KELEMS + i * 130
                                            + (b * NLOC + c2 * 128) * VW),
                                    ap=[[VW, 128], [KVELEMS, CORES], [1, 130]]))
            # exp(bias)^T for both heads: [128 kv, KC*NLOC] from biasT_dram
            ebt = [None, None]
            for parity in range(2):
                et = ebt_pool.tile([128, KC * NLOC], bf16, name="ebt")
                ebt[parity] = et
                nc.scalar.dma_start(
                    out=et[:],
                    in_=bass.AP(tensor=BTD_T,
                                offset=bTd.offset + (2 * i + parity) * NLOC,
                                ap=[[H * NLOC, 128], [128 * H * NLOC, KC],
                                    [1, NLOC]]))

            po = {}
            for parity in range(2):
                for bp in range(2):
                    p_t = out_psum.tile([65, 512], f32, name="po")
                    po[parity, bp] = p_t
                    nc.tensor.matmul(p_t[:, :], zeros65[:], qT[i][:, 0:512],
                                     start=True, stop=False,
                                     skip_group_check=True)

            ae_ring = {}
            for c in range(KC + 1):
                for parity in range(2):
                    if c < KC:
                        ps = sim_psum.tile([128, B * NLOC], f32, name="simps")
                        for b in range(B):
                            nc.tensor.matmul(
                                ps[:, b * NLOC:(b + 1) * NLOC],
                                kt[parity * 64:parity * 64 + 64,
                                   b * N + c * 128:b * N + (c + 1) * 128],
                                qT[i][parity * 64:parity * 64 + 64,
                                      b * NLOC:(b + 1) * NLOC],
                                start=True, stop=True,
                                tile_position=(parity * 64, 0))
                        ae = ae_pool.tile([128, B * NLOC], bf16, name="ae")
                        nc.scalar.activation(ae[:], ps[:], AF.Exp)
                        ebs = ebt[parity][:, c * NLOC:(c + 1) * NLOC]
                        bcast = bass.AP(tensor=ebs.tensor, offset=ebs.offset,
                                        ap=[ebs.ap[0], [0, B], [1, NLOC]])
                        ae3 = ae[:].rearrange("p (b q) -> p b q", b=B)
                        nc.vector.tensor_tensor(out=ae3, in0=ae3, in1=bcast,
                                                op=ALU.mult)
                        ae_ring[c, parity] = ae
                    if c >= 1:
                        cp = c - 1
                        aep = ae_ring.pop((cp, parity))
                        for b in range(B):
                            nc.tensor.matmul(
                                po[parity, b // 2][:, (b % 2) * NLOC:
                                                   ((b % 2) + 1) * NLOC],
                                vt[:, b * KC * 130 + cp * 130 + parity * 65:
                                   b * KC * 130 + cp * 130 + parity * 65 + 65],
                                aep[:, b * NLOC:(b + 1) * NLOC],
                                start=False, stop=(cp == KC - 1),
                                skip_group_check=True)

            # evacuate pair outputs + denominators
            den_row = den_pool.tile([65, 2 * ROWS], f32, name="den")
            stag = stag_pool.tile([64, ROWS], bf16, name="stag")
            for parity in range(2):
                for bp in range(2):
                    p_t = po[parity, bp]
                    if parity == 0:
                        nc.vector.tensor_copy(
                            asm[i][0:64, bp * 512:(bp + 1) * 512], p_t[0:64, :])
                    else:
                        nc.vector.tensor_copy(
                            stag[0:64, bp * 512:(bp + 1) * 512], p_t[0:64, :])
                    nc.vector.tensor_copy(
                        den_row[64:65, parity * ROWS + bp * 512:
                                parity * ROWS + (bp + 1) * 512],
                        p_t[64:65, :])
            nc.scalar.dma_start(asm[i][64:128, :], stag[0:64, :])
            for parity in range(2):
                nc.scalar.dma_start(
                    denom_asm[2 * i + parity:2 * i + parity + 1, :],
                    den_row[64:65, parity * ROWS:(parity + 1) * ROWS])

        attn_stack.close()

        # ------------------------------------------------------------------
        # Phase 6: normalize by softmax denominator
        fin_stack = contextlib.ExitStack()
        rb_psum = fin_stack.enter_context(
            tc.tile_pool(name="rbp", bufs=2, space="PSUM"))
        rb_pool = fin_stack.enter_context(tc.tile_pool(name="rbs", bufs=2))
        recip = cpool.tile([H, ROWS], f32, name="recip")
        nc.vector.reciprocal(recip[:], denom_asm[:])
        for i in range(8):
            ps = rb_psum.tile([128, ROWS], f32, name="rbp")
            for nh in range(2):
                nc.tensor.matmul(ps[:, nh * 512:(nh + 1) * 512],
                                 sel[:, i * 128:(i + 1) * 128],
                                 recip[:, nh * 512:(nh + 1) * 512],
                                 start=True, stop=True)
            rbs = rb_pool.tile([128, ROWS], f32, name="rbs")
            nc.vector.tensor_copy(rbs[:], ps[:])
            nc.vector.tensor_mul(asm[i][:], asm[i][:], rbs[:])

        # ------------------------------------------------------------------
        # Phase 7: output projection
        wo_pool = fin_stack.enter_context(tc.tile_pool(name="wo", bufs=1))
        wout_bf = []
        for t in range(8):
            wot = wo_pool.tile([128, DIM], bf16, name=f"wo{t}")
            nc.gpsimd.dma_start(wot[:], wout_in[t * 128:(t + 1) * 128, :])
            wout_bf.append(wot)
        f_psum = fin_stack.enter_context(
            tc.tile_pool(name="fp", bufs=2, space="PSUM"))
        o_pool = fin_stack.enter_context(tc.tile_pool(name="osb", bufs=3))
        for mi in range(8):
            ps = f_psum.tile([128, DIM], f32, name="fp")
            for ki in range(8):
                for nh in range(2):
                    nc.tensor.matmul(ps[:, nh * 512:(nh + 1) * 512],
                                     asm[ki][:, mi * 128:(mi + 1) * 128],
                                     wout_bf[ki][:, nh * 512:(nh + 1) * 512],
                                     start=(ki == 0), stop=(ki == 7))
            ot = o_pool.tile([128, DIM], f32, name="ot")
            nc.vector.tensor_copy(ot[:], ps[:])
            nc.sync.dma_start(out_ext[mi * 128:(mi + 1) * 128, :], ot[:])

        fin_stack.close()

    nc.finalize()
    return nc


def _get_nc():
    if "nc" not in _CACHE:
        _CACHE["nc"] = _build_nc()
    return _CACHE["nc"]


def kernel(x, attn_bias, ln_g, ln_b, Wq, Wkv, Wout):
    from concourse import bass_utils

    nc = _get_nc()
    x = np.asarray(x, dtype=np.float32)
    attn_bias = np.asarray(attn_bias, dtype=np.float32)
    in_maps = []
    for r in range(CORES):
        in_maps.append({
            "x": np.ascontiguousarray(
                x[:, r * NLOC:(r + 1) * NLOC, :]).reshape(ROWS, DIM),
            "attn_bias": np.ascontiguousarray(
                attn_bias[:, r * NLOC:(r + 1) * NLOC, :]),
            "ln_g": np.asarray(ln_g, dtype=np.float32),
            "ln_b": np.asarray(ln_b, dtype=np.float32),
            "Wq": np.asarray(Wq, dtype=np.float32),
            "Wkv": np.asarray(Wkv, dtype=np.float32),
            "Wout": np.asarray(Wout, dtype=np.float32),
        })
    res = bass_utils.run_bass_kernel_spmd(nc, in_maps, core_ids=list(range(CORES)))
    out = np.empty((B, N, DIM), dtype=np.float32)
    for r in range(CORES):
        out[:, r * NLOC:(r + 1) * NLOC, :] = \
            res.results[r]["out"].reshape(B, NLOC, DIM)
    return out
